# revision 47
# baseline (speedup 1.0000x reference)
"""Masked dot-product attention on 8 TRN2 NeuronCores.

Math (per batch b):
    S = Q @ K^T / sqrt(64)                    [SQ, SK]
    S[:, k >= vl_b] = -1e6; A = softmax(S)    (masked cols -> weight 0)
    O = A @ V                                 [SQ, 64]

Device strategy (per core, SPMD — identical instruction stream):
  * scores are computed transposed: S_T[k, q] = sum_d K[k,d] Q[q,d]
    via matmul(lhsT=K^T tile [64,128], rhs=Q^T chunk [64,512]).
  * no max-subtraction: |S/8| <= ~6 so exp never overflows; the
    reference's masked lanes underflow to exactly 0 in fp32, we instead
    zero V rows (host-side) so masked keys contribute 0 to both
    numerator and denominator — identical result, zero device masking
    cost.
  * denominator via ones-column appended to V (host-side):
    O_aug^T[65, q] = sum_k V_aug[k, :]^T * exp(S_T[k, q]) accumulated in
    PSUM over k-tiles; row 64 is the softmax denominator.
  * host does final divide + transpose (tiny), so the device never
    needs cross-partition broadcasts.
  * matmul operands are fp16 (PE streams 2-byte dtypes at full rate;
    4-byte f32r measured 2.6x slower). PSUM accumulation stays fp32.

Work scheduling: the host knows valid_lens at compile time, so each core
receives a host-packed list of (q-chunk "slot", k-tile "unit") work items
covering only k < vl. All cores run the same program shape (same slot/unit
counts, compile-time constants); per-core differences live entirely in the
packed input data. Cores with fewer real k-tiles get padding units whose
V_aug is all-zero (contributes nothing).
"""

import functools
import math

import numpy as np

B, SQ, SK, D = 16, 2048, 2048, 64
NCORES = 8
KT = 128          # k rows per unit (one matmul stationary tile)
QCH = 1024        # q columns per slot
NSLOTS_TOTAL = B * (SQ // QCH)   # 32 slot-items across all cores
SLOTS_PER_CORE = NSLOTS_TOTAL // NCORES  # 4
VA_W = D + 1      # V columns + ones column
VA_P = KT         # V_aug padded to 128 cols: full-width mm2 keeps the PE
                  # array's HAM activity high (half-idle arrays throttle the
                  # clock to 1.2 GHz) and enables fast weight load
PW = KT + 2 * VA_P  # merged pair row width: K^T pair cols + 2x padded V_aug

_last_results = None  # stashed BassKernelResults for test.py introspection


def _nkt(vl: int) -> int:
    return max(1, min(SK // KT, math.ceil(vl / KT)))


def _make_schedule(vl: np.ndarray, full: bool = False):
    """Assign the 32 (batch, q-half) slot-items to 8 cores, balanced by
    k-tile count. An item may be SPLIT across slots/cores (partial-k
    attention sums are additive; the host sums partial outputs before
    dividing), which lets slot sizes drop below their group max with the
    overflow going to shared spill slots.

    Returns (slot_sizes, assign): slot_sizes[s] is the compile-time unit
    count of slot s (identical on every core); assign[core][s] is
    (batch, half, k_tile_start, n_real_ktiles) or None (pure padding)."""
    w = [SK // KT if full else _nkt(int(vl[b])) for b in range(B)]
    items = sorted(((b, h) for b in range(B) for h in range(SQ // QCH)),
                   key=lambda t: -w[t[0]])
    ngroups = len(items) // NCORES  # 4
    groups = [items[NCORES * s : NCORES * s + NCORES] for s in range(ngroups)]
    gmax = [max(w[b] for b, _ in g) for g in groups]
    gmin = [min(w[b] for b, _ in g) for g in groups]

    def evaluate(p):
        leftovers = []  # (len, batch, half, k_start)
        for s, g in enumerate(groups):
            for b, h in g:
                if w[b] > p[s]:
                    leftovers.append((w[b] - p[s], b, h, p[s]))
        leftovers.sort(key=lambda t: -t[0])
        spares = []
        for i in range(0, len(leftovers), NCORES):
            spares.append(leftovers[i : i + NCORES])
        spare_sizes = [chunk[0][0] for chunk in spares]
        return sum(p) + sum(spare_sizes), spares, spare_sizes

    import itertools
    best = None
    ranges = [range(gmin[s], gmax[s] + 1) for s in range(ngroups)]
    # keep the search tractable: only consider the top few reductions
    ranges = [r if len(r) <= 8 else range(gmax[s] - 7, gmax[s] + 1)
              for s, r in zip(range(ngroups), ranges)]
    for p in itertools.product(*ranges):
        total, spares, spare_sizes = evaluate(list(p))
        # each slot adds a pipeline-boundary stall worth ~1.2 units
        cost = total + 1.2 * (len(p) + len(spares))
        if best is None or cost < best[0]:
            best = (cost, list(p), spares, spare_sizes)
    _, p, spares, spare_sizes = best

    slot_sizes = list(p) + spare_sizes
    assign = [[None] * len(slot_sizes) for _ in range(NCORES)]
    for s, g in enumerate(groups):
        for c, (b, h) in enumerate(g):
            assign[c][s] = (b, h, 0, min(w[b], p[s]))
    for k, chunk in enumerate(spares):
        for c, (ln, b, h, k_start) in enumerate(chunk):
            assign[c][ngroups + k] = (b, h, k_start, ln)
    # order slots largest-first so the kernel tail is the smallest slot
    order = sorted(range(len(slot_sizes)), key=lambda s: -slot_sizes[s])
    slot_sizes = [slot_sizes[s] for s in order]
    assign = [[a[s] for s in order] for a in assign]
    return tuple(slot_sizes), assign


@functools.lru_cache(maxsize=4)
def _build_program(slot_sizes: tuple):
    """Build + schedule the SPMD Bass program for the given slot shape."""
    import concourse.bacc as bacc
    import concourse.mybir as mybir
    import concourse.tile as tile

    n_units = sum(slot_sizes)
    f32 = mybir.dt.float32
    f16 = mybir.dt.float16

    nc = bacc.Bacc(
        "TRN2",
        target_bir_lowering=False,
        debug=False,
        enable_asserts=False,
        num_devices=NCORES,
    )
    n_pairs = sum((u + 1) // 2 for u in slot_sizes)  # slot-local pairing
    n_slots = len(slot_sizes)
    qtd = nc.dram_tensor("qtd", [n_slots, KT, QCH], f16, kind="ExternalInput")
    uin = nc.dram_tensor("uin", [n_pairs, KT, PW], f16, kind="ExternalInput")
    o = nc.dram_tensor("o", [n_slots, VA_W, QCH], f32, kind="ExternalOutput")

    with tile.TileContext(nc) as tc:
        with (
            tc.tile_pool(name="qpool", bufs=3) as qpool,
            tc.tile_pool(name="upool", bufs=8) as upool,
            tc.tile_pool(name="ptpool", bufs=4) as ptpool,
            tc.tile_pool(name="opool", bufs=2) as opool,
            tc.tile_pool(name="scpool", bufs=1, space="PSUM") as scpool,
            tc.tile_pool(name="accpool", bufs=1, space="PSUM") as accpool,
        ):
            # Per pair of k-tile units (A, B): the 4 mm1 matmuls are emitted
            # interleaved (A-c0, B-c0, A-c1, B-c1) on PE row groups h0/h64 so
            # the two 64-deep contractions execute CONCURRENTLY in the array.
            # This both halves mm1 time and keeps array activity high enough
            # for the HAM clock gate to run the PE at full clock (a K=64
            # half-array stream alone stays throttled at 1.2 GHz).
            #
            # PE queue order is pinned to
            #   ... mm1-pair(p) -> mm2-pair(p-1) -> mm1-pair(p+1) ...
            # so the previous pair's mm2 fills the exp latency. Score tiles
            # rotate through 3 single-buffered PSUM tags (6 banks, +2 for the
            # accumulator = all 8), giving mm1 three units of WAR slack
            # against exp.
            scale = 1.0 / math.sqrt(D)
            exp_f = mybir.ActivationFunctionType.Exp
            # Dummy exp with no dependencies: pulls the ~2.7us ACT table
            # load into the DMA-priming phase instead of the first real exp.
            warm = qpool.tile([1, 8], f32, name="warm", tag="warm")
            nc.vector.memset(warm, 0.0)
            nc.scalar.activation(warm, warm, exp_f, scale=1.0)
            pending = []      # mm2 calls of the previous pair (emitted,
                              # ordering deferred until next pair's mm1s)
            prev_mm2_last = None  # last mm2 of the pair before that
            gu = 0   # unit counter (sc-tag rotation)
            p_idx = 0  # global pair counter (uin index)
            for s, nu in enumerate(slot_sizes):
                # Q^T chunk duplicated into both partition halves (h64 stream)
                qt = qpool.tile([KT, QCH], f16)
                nc.sync.dma_start(out=qt, in_=qtd[s])
                acc = accpool.tile([KT, QCH], f32)
                for jp in range((nu + 1) // 2):
                    ump = upool.tile([KT, PW], f16)
                    nc.sync.dma_start(out=ump, in_=uin[p_idx])
                    p_idx += 1
                    # A lone unit still gets a dummy row-group-B partner for
                    # mm1 (zero V_aug, no exp/mm2): a half-array matmul
                    # stream drops the HAM activity metric and re-throttles
                    # the PE clock to 1.2 GHz.
                    lone = 2 * jp + 1 >= nu
                    units = []
                    for half in (0, 1):
                        j = 2 * jp + half
                        real = not (lone and half == 1)
                        rows = slice(0, D) if half == 0 else slice(D, KT)
                        units.append((
                            j,
                            real,
                            ump[rows, 0:KT],                     # K^T tile
                            qt[rows, :],                          # Q^T stream
                            ump[:, KT + half * VA_P : KT + (half + 1) * VA_P],
                            scpool.tile([KT, QCH], f32, name=f"sc_{gu}_{half}",
                                        tag=f"sc{(gu + half) % 3}"),
                            ptpool.tile([KT, QCH], f16, name=f"pt_{gu}_{half}",
                                        tag=f"pt{half}"),
                        ))
                    mm1 = []
                    nchunk = QCH // 512
                    for c in range(nchunk):
                        for j, real, kt_t, qt_h, va_t, sc, pt in units:
                            mm1.append(nc.tensor.matmul(
                                sc[:, c * 512 : (c + 1) * 512],
                                lhsT=kt_t,
                                rhs=qt_h[:, c * 512 : (c + 1) * 512],
                                start=True,
                                stop=True,
                            ))
                            # emit each unit's exp right after its last mm1
                            # chunk so its ACT-queue wait lands per-exp (a
                            # trailing wait would gate exp-A on B's matmuls)
                            if c == nchunk - 1 and real:
                                nc.scalar.activation(pt, sc, exp_f, scale=scale)
                    if prev_mm2_last is not None:
                        tile.add_dep_helper(mm1[0].ins, prev_mm2_last.ins,
                                            False, "pe order")
                    for a, b in zip(mm1, mm1[1:]):
                        tile.add_dep_helper(b.ins, a.ins, False, "pe order")
                    for mm2 in pending:
                        tile.add_dep_helper(mm2.ins, mm1[-1].ins, False,
                                            "mm2 after next pair's mm1")
                    prev_mm2_last = pending[-1] if pending else prev_mm2_last
                    pending = []
                    for j, real, kt_t, qt_h, va_t, sc, pt in units:
                        if not real:
                            continue
                        for c in range(QCH // 512):
                            pending.append(nc.tensor.matmul(
                                acc[:, c * 512 : (c + 1) * 512],
                                lhsT=va_t,
                                rhs=pt[:, c * 512 : (c + 1) * 512],
                                start=(j == 0),
                                stop=(j == nu - 1),
                            ))
                    for a, b in zip(pending, pending[1:]):
                        tile.add_dep_helper(b.ins, a.ins, False, "pe order")
                    gu += 2
                # copy + store per 512-col half so the first half streams out
                # while the slot's last mm2 still writes the second half.
                # For the final slot the copy is on the critical tail: split
                # it across DVE and the (now idle) scalar engine.
                o_sb = opool.tile([VA_W, QCH], f32)
                last = s == len(slot_sizes) - 1
                for c in range(QCH // 512):
                    src = acc[0:VA_W, c * 512 : (c + 1) * 512]
                    dst = o_sb[:, c * 512 : (c + 1) * 512]
                    if last and c == 1:
                        nc.scalar.activation(
                            dst, src, mybir.ActivationFunctionType.Copy)
                    else:
                        nc.vector.tensor_copy(dst, src)
                    nc.sync.dma_start(out=o[s, :, c * 512 : (c + 1) * 512],
                                      in_=o_sb[:, c * 512 : (c + 1) * 512])
    nc.compile()
    return nc


def _pack_inputs(queries, keys, values, vl, slot_sizes, assign):
    """Build each core's packed device inputs per its schedule (mirrors the
    device program's slot-local pairing exactly)."""
    n_pairs = sum((u + 1) // 2 for u in slot_sizes)
    n_slots = len(slot_sizes)
    qT = np.ascontiguousarray(queries.transpose(0, 2, 1).astype(np.float16))
    kT = keys.astype(np.float16)  # [B, SK, D] row-major, sliced per k-tile
    in_maps = []
    for c in range(NCORES):
        qtd = np.zeros((n_slots, KT, QCH), np.float16)
        uin = np.zeros((n_pairs, KT, PW), np.float16)
        p_idx = 0
        for s, nu in enumerate(slot_sizes):
            if assign[c][s] is None:
                p_idx += (nu + 1) // 2
                continue  # pure-padding slot: all-zero inputs contribute 0
            b, h, ks, w = assign[c][s]
            qtd[s, :D] = qT[b, :, h * QCH : (h + 1) * QCH]
            qtd[s, D:KT] = qtd[s, :D]  # duplicate for the h64 row half
            nvalid = int(vl[b])
            for jp in range((nu + 1) // 2):
                for half in (0, 1):
                    # a lone unit's B half is a dummy mm1 partner (device
                    # skips its exp/mm2): real K data keeps array activity up
                    j = min(2 * jp + half, nu - 1)
                    t = ks + min(j, w - 1)  # padding units replay a k-tile
                    rows = slice(0, D) if half == 0 else slice(D, KT)
                    uin[p_idx, rows, :KT] = kT[b, t * KT : (t + 1) * KT, :].T
                    if j < w and not (half == 1 and 2 * jp + 1 >= nu):
                        k0 = t * KT
                        nv = min(max(nvalid - k0, 0), KT)
                        col0 = KT + half * VA_P
                        uin[p_idx, :nv, col0 : col0 + D] = values[b, k0 : k0 + nv, :]
                        uin[p_idx, :nv, col0 + D] = 1.0
                    # padding units leave V_aug zero -> contribute nothing
                p_idx += 1
        in_maps.append({"qtd": qtd, "uin": uin})
    return in_maps


def kernel(queries, keys, values, valid_lens, _full=False, _trace=False):
    global _last_results
    from concourse.bass_utils import run_bass_kernel_spmd

    queries = np.ascontiguousarray(np.asarray(queries, dtype=np.float32))
    keys = np.ascontiguousarray(np.asarray(keys, dtype=np.float32))
    values = np.ascontiguousarray(np.asarray(values, dtype=np.float32))
    vl = np.asarray(valid_lens).astype(np.int64).reshape(B)

    slot_sizes, assign = _make_schedule(vl, full=_full)
    nc = _build_program(slot_sizes)
    in_maps = _pack_inputs(queries, keys, values, vl, slot_sizes, assign)

    kwargs = {"trace": True} if _trace else {}
    res = run_bass_kernel_spmd(nc, in_maps, core_ids=list(range(NCORES)), **kwargs)
    _last_results = res

    # Sum partial (numerator, denominator) contributions per (batch, q-half),
    # then divide once — exact for split items.
    agg = np.zeros((B, SQ // QCH, VA_W, QCH), np.float64)
    for c in range(NCORES):
        o = res.results[c]["o"]  # [n_slots, VA_W, QCH]
        for s in range(len(slot_sizes)):
            if assign[c][s] is None:
                continue
            b, h, _, _ = assign[c][s]
            agg[b, h] += o[s]
    out = np.empty((B, SQ, D), np.float32)
    for b in range(B):
        for h in range(SQ // QCH):
            num = agg[b, h, :D, :]
            den = agg[b, h, D, :]
            out[b, h * QCH : (h + 1) * QCH, :] = (num / den).T.astype(np.float32)
    return out
